# revision 27
# baseline (speedup 1.0000x reference)
"""Trainium2 Bass kernel for CRF loss (nn_CRF_89258010346242).

Strategy (8 NeuronCores, data-parallel over batch, 16 batches/core):

Forward (log-partition) score runs in the exp domain so each recurrence
step is one small PE matmul (stationary exp(transitions)) plus one DVE
elementwise multiply with exp(emissions), scaled per step by exp(-C).
The constant normalizer C keeps f32 magnitudes bounded (|log state| <
~25) so no runtime renormalization is needed; the final log corrects by
len[b]*C.

The 511-step serial chain is split into two concurrent halves:
  - fwd chain t=1..256 computing alpha_t (each step writes a fresh
    [50,16] tile; background copies consolidate them into [50,512]
    history chunks); batches with len<=256 are captured afterwards by a
    one-hot-selected reduce of the history against exp(trans[:, STOP]).
  - bwd chain t=511..256 computing the backward message w_t, with
    exp(trans[:,STOP]) injected via a tiny accumulated matmul at
    t=len[b]-1 (zero columns propagate for short batches), meeting the
    fwd chain at t=256 with an inner product.

Gold score: two indirect-DMA gathers (emission scores and transition
scores) with masked indices pointing at a zero slot, then reduces.

Hardware constraint honored throughout: every TPB instruction can carry
at most ONE sync-wait; waits on the same semaphore merge.  Cross-engine
dependencies are funneled through single-wait "probe" copies, PE dummy
matmuls pre-observe the ACT/DMA semaphores, and chain outputs go to
fresh tiles so no same-engine WAW waits are emitted.

Each core emits one f32 partial (forward_partial - gold_partial); the
host sums the 8 partials.
"""

import numpy as np

T = 50            # tag size (incl START=48, STOP=49)
START, STOP = 48, 49
S = 512           # sequence length
B = 128           # total batch
NCORES = 8
BC = B // NCORES  # batches per core = 16
MID = 256         # fwd/bwd split point
C = 4.4           # constant per-step normalizer (log domain)

EE_CHUNK = 1024   # ee/em chunk width (64 timesteps)
N_EE = (S * BC) // EE_CHUNK            # 8
AH_CHUNK = 512    # alpha-history chunk width (32 slots)
N_AH = (MID * BC) // AH_CHUNK          # 8
EMZ = T * S * BC                       # zero-slot index in flat emissions
TRZ = T * T                            # zero-slot index in flat transitions


def build_program(report="loss"):
    import contextlib

    import concourse.bass as bass
    import concourse.mybir as mybir
    import concourse.tile as tile

    dt = mybir.dt
    f32 = dt.float32
    Alu = mybir.AluOpType
    Act = mybir.ActivationFunctionType
    Axis = mybir.AxisListType

    EXPNC = float(np.exp(np.float32(-C)))

    nc = bass.Bass()

    em_d = nc.declare_dram_parameter("em", [EMZ + BC, 1], f32, isOutput=False)
    trflat_d = nc.declare_dram_parameter("trflat", [TRZ + 1, 1], f32, isOutput=False)
    tr_d = nc.declare_dram_parameter("tr", [T, T], f32, isOutput=False)
    trT_d = nc.declare_dram_parameter("trT", [T, T], f32, isOutput=False)
    etscol_d = nc.declare_dram_parameter("etscol", [1, T], f32, isOutput=False)
    ohE_d = nc.declare_dram_parameter("ohE", [1, (S - MID) * BC], f32, isOutput=False)
    ohFb_d = nc.declare_dram_parameter("ohFb", [T, MID * BC], f32, isOutput=False)
    lenc_d = nc.declare_dram_parameter("lenc", [1, BC], f32, isOutput=False)
    idxt_d = nc.declare_dram_parameter("idxt", [128, 65], dt.int32, isOutput=False)
    ohm_d = nc.declare_dram_parameter("ohm", [T, S * BC], f32, isOutput=False)
    out_d = nc.declare_dram_parameter("out", [1, 1], f32, isOutput=True)

    # flat emissions viewed as [T, S*BC] for the chunk loads
    em_mat = em_d[0:EMZ, 0:1].rearrange("(a b) c -> a (b c)", b=S * BC)

    # Tile's kernel-tail drain aggregates every outstanding semaphore into
    # ONE instruction, overflowing its sync-wait slots for kernels that touch
    # many queues.  Pre-absorb the global clock across several drains with at
    # most 3 waits each; the original drain then needs none.
    if not getattr(tile.TileContext, "_crf_drain_patched", False):
        _orig_dab = tile.TileContext._drain_and_barrier
        from concourse.tile import ScopedClock as _SC

        def _patched_dab(self, tick_clock, wait_clock):
            drain_inst = self.nc.sync.drain()
            wait_clock.add_sem_waits(
                drain_inst.ins, _SC({None: tick_clock.global_clock}))
            si = drain_inst.ins.sync_info
            waits = list(si.on_wait) if si is not None else []
            MAXW = 1
            if len(waits) > MAXW:
                si.on_wait = waits[:MAXW]
                rest = waits[MAXW:]
                while rest:
                    d2 = self.nc.sync.drain()
                    d2.ins.sync_info = mybir.SyncInfo(
                        on_wait=rest[:MAXW], on_update=[])
                    rest = rest[MAXW:]
            # barrier + semaphore cleanup, verbatim from the original
            self.nc.all_engine_barrier()
            popped = self.nc._tile_sem_poison_stack.pop()
            assert popped is self._sem_poison
            self.nc.clear_and_free_semaphores(
                list(self.sems.allocated().values()))
            self.nc.all_engine_barrier()

        tile.TileContext._drain_and_barrier = _patched_dab
        tile.TileContext._crf_drain_patched = True

    with tile.TileContext(nc) as tc:
        ctx = contextlib.ExitStack()
        with ctx:
            persist = ctx.enter_context(tc.tile_pool(name="persist", bufs=1))
            em_pool = ctx.enter_context(tc.tile_pool(name="em", bufs=1))
            fpsum = ctx.enter_context(tc.tile_pool(name="fpsum", bufs=2, space="PSUM"))
            bpsum = ctx.enter_context(tc.tile_pool(name="bpsum", bufs=2, space="PSUM"))
            xpsum = ctx.enter_context(tc.tile_pool(name="xpsum", bufs=1, space="PSUM"))

            # ---- persistent tiles ----
            ee = [persist.tile([T, EE_CHUNK], f32, tag=f"ee{c}", name=f"ee{c}")
                  for c in range(N_EE)]
            ah = [persist.tile([T, AH_CHUNK], f32, tag=f"ah{c}", name=f"ah{c}")
                  for c in range(N_AH)]
            fa = [persist.tile([T, BC], f32, tag=f"fa{t}", name=f"fa{t}")
                  for t in range(MID)]
            uw = [persist.tile([T, BC], f32, tag=f"uw{t}", name=f"uw{t}")
                  for t in range(MID)]
            ohFb = [persist.tile([T, AH_CHUNK], f32, tag=f"ohFb{j}", name=f"ohFb{j}")
                    for j in range(N_AH)]
            ohE_tiles = [persist.tile([1, EE_CHUNK], f32, tag=f"ohE{c}", name=f"ohE{c}")
                         for c in range(4)]
            et = persist.tile([T, T], f32, tag="et", name="et")
            etT = persist.tile([T, T], f32, tag="etT", name="etT")
            injrow = persist.tile([1, T], f32, tag="injrow", name="injrow")
            ones = persist.tile([128, 1], f32, tag="ones", name="ones")
            lenc = persist.tile([1, BC], f32, tag="lenc", name="lenc")
            idxt = persist.tile([128, 65], dt.int32, tag="idxt", name="idxt")
            gtr = persist.tile([128, 65], f32, tag="gtr", name="gtr")
            gred = persist.tile([128, 1], f32, tag="gred", name="gred")
            red = persist.tile([T, BC], f32, tag="red", name="red")
            meetP = persist.tile([T, BC], f32, tag="meetP", name="meetP")
            logtot = persist.tile([1, BC], f32, tag="logtot", name="logtot")
            final16 = persist.tile([1, BC], f32, tag="final16", name="final16")
            fpart = persist.tile([1, 1], f32, tag="fpart", name="fpart")
            outsb = persist.tile([1, 1], f32, tag="outsb", name="outsb")
            trs = persist.tile([T, T], f32, tag="trs", name="trs")
            trTs = persist.tile([T, T], f32, tag="trTs", name="trTs")
            etsraw = persist.tile([1, T], f32, tag="etsraw", name="etsraw")
            probe = persist.tile([1, 128], f32, tag="probe", name="probe")

            probe_n = [0]

            def dve_probe(src_ap):
                """One-wait TensorCopy absorbing a foreign semaphore into
                DVE's observed clock (each probe writes a fresh column)."""
                n = probe_n[0]
                probe_n[0] += 1
                nc.vector.tensor_copy(probe[0:1, n:n + 1], src_ap)

            # ---- small input DMAs + weight prep ----
            nc.sync.dma_start(trs[:], tr_d[:])
            nc.sync.dma_start(trTs[:], trT_d[:])
            nc.sync.dma_start(etsraw[:], etscol_d[:])
            nc.sync.dma_start(lenc[:], lenc_d[:])
            nc.gpsimd.dma_start(idxt[:], idxt_d[:])
            nc.scalar.activation(et[:], trs[:], Act.Exp)
            nc.scalar.activation(etT[:], trTs[:], Act.Exp)
            nc.scalar.activation(injrow[:], etsraw[:], Act.Exp)
            nc.vector.memset(ones[:], 1.0)
            nc.vector.memset(red[:], 0.0)
            ered = persist.tile([T, 1], f32, tag="ered", name="ered")
            nc.vector.memset(ered[:], 0.0)
            dve_probe(lenc[0:1, 0:1])

            # ---- em/ee chunk DMAs + exp; alternating ends so both chains
            # start early.  Rotating em slots: the DMA's only wait is the
            # previous exp on ACT. ----
            chunk_order = [0, N_EE - 1, 1, N_EE - 2, 2, N_EE - 3, 3, 4]
            em_tiles = {}
            for c in chunk_order:
                emt = em_pool.tile([T, EE_CHUNK], f32, tag=f"emt{c}", name=f"emt{c}")
                nc.gpsimd.dma_start(emt[:], em_mat[:, c * EE_CHUNK:(c + 1) * EE_CHUNK])
                dve_probe(emt[0:1, 0:1])
                nc.scalar.activation(ee[c][:], emt[:], Act.Exp)
                dve_probe(ee[c][0:1, 0:1])
                em_tiles[c] = emt
            for c in (3, 2, 1, 0):
                nc.gpsimd.dma_start(
                    ohE_tiles[c][:], ohE_d[:, c * EE_CHUNK:(c + 1) * EE_CHUNK])

            # ---- gold-score gathers (background on the DMA engines).
            # Masked/padded indices point at trailing zero slots, so every
            # element is written and no pre-zeroing is needed. ----
            for k in range(65):
                nc.gpsimd.indirect_dma_start(
                    out=gtr[:, k:k + 1], out_offset=None, in_=trflat_d[:],
                    in_offset=bass.IndirectOffsetOnAxis(
                        ap=idxt[:, k:k + 1], axis=0))

            def eesl(t):
                c, o = divmod(t * BC, EE_CHUNK)
                return ee[c][:, o:o + BC]

            def ohEsl(t):
                c, o = divmod((t - MID) * BC, EE_CHUNK)
                return ohE_tiles[c][:, o:o + BC]

            # ---- fwd init: alpha_0 = ee_0 * exp(trans[START, :]) * e^-C ----
            nc.vector.tensor_scalar(fa[0][:], eesl(0), etT[:, START:START + 1],
                                    EXPNC, op0=Alu.mult, op1=Alu.mult)

            # PE pre-observes weight exps, the first ohE chunk and fwd init
            # via bare LdWeights (one wait each, no PSUM side effects).
            nc.tensor.load_weights(lhsT=injrow[0:1, 0:1])
            nc.tensor.load_weights(lhsT=ohE_tiles[3][0:1, 0:1])
            nc.tensor.load_weights(lhsT=fa[0][0:1, 0:1])

            # alpha-history consolidation + capture-select chunks
            cons_jobs = list(range(MID))      # fa[s] -> ah chunk slice
            asel_jobs = list(range(N_AH))

            # emission-gold: (em ⊙ ohm) multiply-reduce, chained accumulator
            EMIT_SUB = 256
            per_chunk = EE_CHUNK // EMIT_SUB
            emit_jobs = [(c, s) for c in chunk_order for s in range(per_chunk)]
            ohm_tiles = {}

            def do_emit_job(c, s):
                if s == 0:
                    omt = persist.tile([T, EE_CHUNK], f32, tag=f"ohm{c}",
                                       name=f"ohm{c}")
                    nc.gpsimd.dma_start(
                        omt[:], ohm_d[:, c * EE_CHUNK:(c + 1) * EE_CHUNK])
                    dve_probe(omt[0:1, 0:1])
                    ohm_tiles[c] = omt
                o = s * EMIT_SUB
                scr = persist.tile([T, EMIT_SUB], f32, tag="scr",
                                   name=f"scr{c}_{s}", bufs=2)
                acc = persist.tile([T, 1], f32, tag=f"eacc{c}_{s}",
                                   name=f"eacc{c}_{s}")
                nc.vector.scalar_tensor_tensor(
                    out=scr[:],
                    in0=em_tiles[c][:, o:o + EMIT_SUB],
                    scalar=1.0,
                    in1=ohm_tiles[c][:, o:o + EMIT_SUB],
                    op0=Alu.mult, op1=Alu.mult,
                    accum_out=acc[:])
                nc.vector.tensor_add(ered[:], ered[:], acc[:])

            def do_cons_job(s):
                c, o = divmod(s * BC, AH_CHUNK)
                nc.vector.tensor_copy(ah[c][:, o:o + BC], fa[s][:])

            def do_asel_job(j):
                nc.gpsimd.dma_start(
                    ohFb[j][:], ohFb_d[:, j * AH_CHUNK:(j + 1) * AH_CHUNK])
                dve_probe(ohFb[j][0:1, 0:1])
                redc = persist.tile([T, BC], f32, tag=f"redc{j}", name=f"redc{j}")
                nc.vector.tensor_mul(ah[j][:], ah[j][:], ohFb[j][:])
                nc.vector.tensor_reduce(
                    out=redc[:],
                    in_=ah[j][:].rearrange("p (t b) -> p b t", b=BC),
                    axis=Axis.X, op=Alu.add)
                nc.vector.tensor_add(red[:], red[:], redc[:])

            f256 = xpsum.tile([T, BC], f32, tag="f256", name="f256")

            u_prev = None
            for i in range(MID):
                tf = i + 1          # fwd timestep: 1..256
                tb = S - 1 - i      # bwd timestep: 511..256

                # ---- fwd step ----
                if tf < MID:
                    pf = fpsum.tile([T, BC], f32, tag="pf", name=f"pf{tf}")
                else:
                    pf = f256
                nc.tensor.matmul(out=pf[:], lhsT=et[:], rhs=fa[tf - 1][:],
                                 start=True, stop=True)
                if tf < MID:
                    nc.vector.scalar_tensor_tensor(
                        fa[tf][:], pf[:], EXPNC, eesl(tf),
                        op0=Alu.mult, op1=Alu.mult)

                # ---- bwd step ----
                pb = bpsum.tile([T, BC], f32, tag="pb", name=f"pb{tb}")
                if i == 0:
                    nc.tensor.matmul(out=pb[:], lhsT=injrow[:], rhs=ohEsl(tb),
                                     start=True, stop=True)
                else:
                    nc.tensor.matmul(out=pb[:], lhsT=etT[:], rhs=u_prev[:],
                                     start=True, stop=False)
                    nc.tensor.matmul(out=pb[:], lhsT=injrow[:], rhs=ohEsl(tb),
                                     start=False, stop=True)
                u_t = uw[i]
                nc.vector.scalar_tensor_tensor(
                    u_t[:], pb[:], EXPNC, eesl(tb), op0=Alu.mult, op1=Alu.mult)
                u_prev = u_t

                # ---- background work interleaved into chain gaps ----
                if i >= 2 and cons_jobs and cons_jobs[0] <= tf - 2:
                    do_cons_job(cons_jobs.pop(0))
                if i % 32 == 30 and asel_jobs and (asel_jobs[0] + 1) * 32 <= tf - 2:
                    do_asel_job(asel_jobs.pop(0))
                if i % 8 == 5 and emit_jobs:
                    do_emit_job(*emit_jobs.pop(0))

            # ---- drain remaining background jobs ----
            while cons_jobs:
                do_cons_job(cons_jobs.pop(0))
            while asel_jobs:
                do_asel_job(asel_jobs.pop(0))
            while emit_jobs:
                do_emit_job(*emit_jobs.pop(0))

            # ---- meet: len>256 contribution ----
            u256 = uw[MID - 1]
            dve_probe(f256[0:1, 0:1])
            nc.vector.tensor_mul(meetP[:], f256[:], u256[:])

            # total_capture[b] = sum_i red[i,b]*et[i,STOP] + sum_i meetP[i,b]
            red_ps = xpsum.tile([1, BC], f32, tag="red_ps", name="red_ps")
            nc.tensor.matmul(out=red_ps[:], lhsT=et[:, STOP:STOP + 1], rhs=red[:],
                             start=True, stop=False)
            nc.tensor.matmul(out=red_ps[:], lhsT=ones[0:T, :], rhs=meetP[:],
                             start=False, stop=True)
            nc.scalar.activation(logtot[:], red_ps[:], Act.Ln)
            nc.vector.tensor_add(final16[:], logtot[:], lenc[:])
            nc.vector.tensor_reduce(out=fpart[:], in_=final16[:],
                                    axis=Axis.X, op=Alu.add)

            # ---- gold score ----
            # the 65 gather DMAs rotate over 8 DMA semaphores; probe the last
            # eight columns to absorb every queue's final tick
            for k in range(57, 65):
                dve_probe(gtr[0:1, k:k + 1])
            nc.vector.tensor_reduce(out=gred[:], in_=gtr[:], axis=Axis.X, op=Alu.add)
            gold_ps = xpsum.tile([1, 1], f32, tag="xps3", name="gold_ps")
            nc.tensor.matmul(out=gold_ps[:], lhsT=ones[:], rhs=gred[:],
                             start=True, stop=False)
            nc.tensor.matmul(out=gold_ps[:], lhsT=ones[0:T, :], rhs=ered[:],
                             start=False, stop=True)

            # ---- loss partial = forward partial - gold partial ----
            dve_probe(gold_ps[0:1, 0:1])
            if report == "loss":
                nc.vector.tensor_sub(outsb[:], fpart[:], gold_ps[:])
            elif report == "fpart":
                nc.vector.tensor_copy(outsb[:], fpart[:])
            elif report == "gold":
                nc.vector.tensor_copy(outsb[:], gold_ps[:])
            elif report == "gred":
                nc.vector.tensor_reduce(out=outsb[:], in_=gred[:].rearrange("p f -> f p"),
                                        axis=mybir.AxisListType.X, op=Alu.add)
            nc.sync.dma_start(out_d[:], outsb[:])

    return nc


def prep_core_inputs(emissions, mask, tags, transitions, core):
    """Host-side layout prep for one core (slicing/transpose/one-hots only)."""
    f32 = np.float32
    bs = slice(core * BC, (core + 1) * BC)
    em = np.asarray(emissions[bs], dtype=f32)          # [BC, S, T]
    mk = np.asarray(mask[bs])                          # [BC, S] bool
    tg = np.asarray(tags[bs]).astype(np.int64)         # [BC, S]
    tr = np.asarray(transitions, dtype=f32)            # [T, T]
    lengths = mk.sum(axis=1).astype(np.int64)          # [BC]

    # em flat: [j, t, b] order plus BC zero-slots at the end
    em_jtb = np.ascontiguousarray(em.transpose(2, 1, 0)).reshape(EMZ)
    em_flat = np.concatenate([em_jtb, np.zeros(BC, f32)]).reshape(EMZ + BC, 1)

    t_idx = np.arange(MID, S)
    ohE = (lengths[None, :] - 1 == t_idx[:, None]).astype(f32) \
        .reshape(1, (S - MID) * BC)

    s_idx = np.arange(MID)
    ohF_row = (lengths[None, :] == s_idx[:, None] + 1).astype(f32).reshape(MID * BC)
    ohFb = np.ascontiguousarray(np.broadcast_to(ohF_row[None, :], (T, MID * BC)))

    lenc = (lengths.astype(f32) * f32(C)).reshape(1, BC)

    # transition-score gather indices (flat into trflat; TRZ = zero slot)
    prev = np.concatenate([np.full((BC, 1), START, tg.dtype), tg[:, :-1]], axis=1)
    idx_flat = (prev * T + tg).T.reshape(S * BC)       # n = t*BC + b
    idx_flat = np.where(mk.T.reshape(S * BC), idx_flat, TRZ)
    end_ids = np.take_along_axis(tg, (lengths - 1)[:, None], axis=1)[:, 0]
    end_idx = end_ids * T + STOP
    idxt = np.full(128 * 65, TRZ, np.int64)
    idxt[:S * BC] = idx_flat
    idxt[S * BC:S * BC + BC] = end_idx
    idxt = idxt.astype(np.int32).reshape(128, 65)

    # emission one-hot: ohm[j, t*BC+b] = (tags[b,t] == j) & mask[b,t]
    j_idx = np.arange(T)
    ohm = ((tg.T[None, :, :] == j_idx[:, None, None]) & mk.T[None, :, :]) \
        .astype(f32).reshape(T, S * BC)

    return {
        "em": em_flat,
        "trflat": np.ascontiguousarray(
            np.concatenate([tr.reshape(TRZ), [f32(0.0)]]).reshape(TRZ + 1, 1)),
        "tr": np.ascontiguousarray(tr),
        "trT": np.ascontiguousarray(tr.T),
        "etscol": np.ascontiguousarray(tr[:, STOP].reshape(1, T)),
        "ohE": ohE,
        "ohFb": ohFb,
        "lenc": lenc,
        "idxt": idxt,
        "ohm": ohm,
    }


_PROGRAM_CACHE = {}


def kernel(emissions, mask, tags, transitions):
    from concourse.bass_utils import run_bass_kernel_spmd

    if "nc" not in _PROGRAM_CACHE:
        _PROGRAM_CACHE["nc"] = build_program()
    nc = _PROGRAM_CACHE["nc"]

    in_maps = [
        prep_core_inputs(emissions, mask, tags, transitions, core)
        for core in range(NCORES)
    ]
    res = run_bass_kernel_spmd(nc, in_maps, list(range(NCORES)))
    total = np.float32(0.0)
    for r in res.results:
        total = np.float32(total + np.float32(r["out"][0, 0]))
    return total
